# revision 21
# baseline (speedup 1.0000x reference)
"""RNN-T JointNetwork kernel for 8x Trainium2 NeuronCores.

reference:
    enc_proj = einsum('btud,jd->btuj', enc_out, W_enc) + b_enc   # (B,T,1,J)
    dec_proj = einsum('btud,jd->btuj', dec_out, W_dec) + b_dec   # (B,1,U,J)
    joint    = tanh(enc_proj + dec_proj)                         # (B,T,U,J)
    out      = einsum('btuj,vj->btuv', joint, W_out) + b_out     # (B,T,U,V)

Strategy: data-parallel over batch B=8 across the 8 cores (one b each).
Per core:
  - enc_projT [J, T] and dec_projT [J, U] via small GEMMs (weights stationary,
    host-pretransposed operands), bias_joint = b_enc+b_dec folded into dec_projT.
  - loop over 32 f-chunks (f = t*U+u, 8 t-values x 64 u = 512 f per chunk):
      jointT[j, f] = tanh(enc_projT[j,t] + dec_projT[j,u])  (DVE bcast-add + ACT tanh)
      outT[v, f]   = W_outT.T @ jointT   (f32r matmuls, PSUM accum over 5 j-tiles,
                     8 v-tiles of 128 partitions each)
      out-stage: PSUM -> SBUF f16 with the per-partition bias b_out[v] fused,
      alternating between ACT (activation Identity+bias) and DVE
      (tensor_scalar_add) so neither engine is near-critical, then contiguous
      DMA of [128v, 512f] to the f16 DRAM output out_T [V, T*U].
Main GEMM in float32r (TF32-like: full-rate streaming, fp32 accumulate);
dec-side projection operands in bf16 (its 64-wide moving dim runs f32r at
1/4 rate but bf16 at full rate). TimelineSim steady state ~277 us/app at
the f32r tensor roofline (PE ~92%% busy). The output lives transposed
[V, T*U] so the final bias is a per-partition scalar fused into the
PSUM->SBUF drain (alternating ACT/DVE), and each chunk's 8 v-tiles go out
in ONE [V, 512] DMA (32 DMAs/app instead of 256 — descriptor-generation
work was a measurable per-app cost on HW inside loops). Device output is
float16 (~2e-4 RMS quantization vs the 2e-2 gate); kernel() converts back
to float32 + transposes on the host.

Empirical HW notes (axon-tunneled TRN2, no NTFF profiler available):
  - bf16 for the FINAL GEMM operands produces garbage on real HW (rel err
    ~1.3) despite passing CoreSim, and timed only ~11us faster — f32r kept.
  - build_program(repeat=N, hw_loop=True, inner_unroll=K) wraps K unrolled
    applications in a hardware For_i loop of N/K iterations: constant
    instruction count with dispatch overhead amortized over N apps.
    K=8 is the sweet spot (K=32's ~61k-instruction body thrashes
    instruction fetch: +100us/app; K=8/16 are fine, K=8 marginally best).
  - Measured best: repeat=1024, inner_unroll=8, single dispatch per burst:
    382.9 us/app (vs 389.1 us baseline, 414-563 us for plain-unrolled
    NEFFs whose per-dispatch overhead scales with NEFF size).
kernel() itself uses repeat=1.
"""

import sys

import numpy as np

if "/opt/trn_rl_repo" not in sys.path:
    sys.path.insert(0, "/opt/trn_rl_repo")

B, T, U = 8, 256, 64
D, J, V = 512, 640, 1024
P = 128
ND, NJ, NV = D // P, J // P, V // P  # 4, 5, 8
TCH = 8  # t-values per f-chunk
NCHUNK = T // TCH  # 32
FCH = TCH * U  # 512 f-positions per chunk
F = T * U  # 16384

_prog_cache = {}


def build_program(repeat=1, hw_loop=False, inner_unroll=1):
    """Build the per-core program.

    repeat > 1 replays the full computation (projections + joint + final
    GEMM + output DMA) that many times inside one NEFF, with weights loaded
    once — used by test.py to amortize per-dispatch transport overhead when
    measuring steady-state per-application HW time. kernel() uses repeat=1.
    hw_loop=True uses a hardware For_i loop for the repeats (constant
    instruction count); hw_loop=False unrolls in Python. inner_unroll
    (hw_loop only) unrolls that many applications inside the loop body to
    amortize the per-iteration all-engine barrier of For_i.
    """
    import concourse.tile as tile
    from concourse import bacc, mybir

    f32 = mybir.dt.float32
    f32r = mybir.dt.float32r
    f16 = mybir.dt.float16
    Tanh = mybir.ActivationFunctionType.Tanh
    Ident = mybir.ActivationFunctionType.Identity

    nc = bacc.Bacc("TRN2", target_bir_lowering=False, debug=False)

    enc_T = nc.dram_tensor("enc_T", [D, T], f32, kind="ExternalInput").ap()
    dec_T = nc.dram_tensor("dec_T", [D, U], f32, kind="ExternalInput").ap()
    w_enc_T = nc.dram_tensor("w_enc_T", [D, J], f32, kind="ExternalInput").ap()
    w_dec_T = nc.dram_tensor("w_dec_T", [D, J], f32, kind="ExternalInput").ap()
    w_out_T = nc.dram_tensor("w_out_T", [J, V], f32, kind="ExternalInput").ap()
    bias_j = nc.dram_tensor("bias_j", [J, 1], f32, kind="ExternalInput").ap()
    bias_v = nc.dram_tensor("bias_v", [V, 1], f32, kind="ExternalInput").ap()
    # Output is stored f16 transposed [V, T*U] (host converts back): f16
    # halves the HBM write traffic and host<->device bytes (~2e-4 RMS err,
    # vs the 2e-2 gate); the [v, f] layout makes b_out a per-partition
    # scalar so the bias-add fuses into the PSUM->SBUF drain.
    out = nc.dram_tensor("out_T", [V, F], f16, kind="ExternalOutput").ap()

    with tile.TileContext(nc) as tc:
        with (
            tc.tile_pool(name="const", bufs=1) as constp,
            tc.tile_pool(name="proj", bufs=1) as projp,
            tc.tile_pool(name="pre", bufs=6) as prep,
            tc.tile_pool(name="joint", bufs=10) as jointp,
            tc.tile_pool(name="osb", bufs=5) as osbp,
            tc.tile_pool(name="ps", bufs=8, space="PSUM") as psp,
        ):
            # ---- load weights / inputs (one-time) ----
            # f32r matmul operands must be written by a rounding producer
            # (BIR verifier) — stage DMA loads in f32 then round-copy to f32r.
            def load_round(shape, dram_ap, tag):
                stg = constp.tile(shape, f32, tag=f"stage_{tag}")
                nc.sync.dma_start(out=stg[:], in_=dram_ap)
                t_ = constp.tile(shape, f32r, tag=tag)
                nc.vector.tensor_copy(t_[:], stg[:])
                return t_

            bf16 = mybir.dt.bfloat16

            def load_cast(shape, dram_ap, tag, dtype):
                stg = constp.tile(shape, f32, tag=f"stage_{tag}")
                nc.sync.dma_start(out=stg[:], in_=dram_ap)
                t_ = constp.tile(shape, dtype, tag=tag)
                nc.vector.tensor_copy(t_[:], stg[:])
                return t_

            # Final-GEMM operands stay f32r: bf16 operands produced garbage
            # on real HW (rel err ~1.3; fine in CoreSim — likely a 16-bit
            # weight-load layout subtlety) and timed only ~11us faster.
            w_out_sb = [
                load_round([P, V], w_out_T[jt * P : (jt + 1) * P, :], f"wout{jt}")
                for jt in range(NJ)
            ]
            # dec-side projection operands in bf16: the dec GEMM's moving dim
            # is only U=64 (<256), where f32r drops to 1/4 rate but bf16
            # stays full rate. bf16 rounding of dec_out/W_dec adds ~0.1%
            # error to dec_proj, far inside the 2e-2 gate.
            enc_sb, dec_sb, wenc_sb, wdec_sb = [], [], [], []
            for dt_ in range(ND):
                sl = slice(dt_ * P, (dt_ + 1) * P)
                enc_sb.append(load_round([P, T], enc_T[sl, :], f"enc{dt_}"))
                dec_sb.append(load_cast([P, U], dec_T[sl, :], f"dec{dt_}", bf16))
                wenc_sb.append(load_round([P, J], w_enc_T[sl, :], f"wenc{dt_}"))
                wdec_sb.append(
                    load_cast([P, J], w_dec_T[sl, :], f"wdec{dt_}", bf16)
                )
            bj_sb = constp.tile([P, NJ], f32, tag="bj")
            nc.sync.dma_start(
                out=bj_sb[:],
                in_=bias_j.rearrange("(jt p) one -> p (jt one)", p=P),
            )
            bv_sb = constp.tile([P, NV], f32, tag="bv")
            nc.sync.dma_start(
                out=bv_sb[:],
                in_=bias_v.rearrange("(vt p) one -> p (vt one)", p=P),
            )

            # ---- repeated body: projections + joint + final GEMM ----
            if hw_loop and repeat > 1:
                assert repeat % inner_unroll == 0
                with tc.For_i(0, repeat // inner_unroll):
                    for _inner in range(inner_unroll):
                        run_body(nc, tc, projp, prep, jointp, osbp, psp,
                                 enc_sb, dec_sb, wenc_sb, wdec_sb, w_out_sb,
                                 bj_sb, bv_sb, out, f32, f32r, f16, Tanh,
                                 Ident)
            else:
                for _rep in range(repeat):
                    run_body(nc, tc, projp, prep, jointp, osbp, psp, enc_sb,
                             dec_sb, wenc_sb, wdec_sb, w_out_sb, bj_sb,
                             bv_sb, out, f32, f32r, f16, Tanh, Ident)
    nc.compile()
    return nc


def run_body(nc, tc, projp, prep, jointp, osbp, psp, enc_sb, dec_sb,
             wenc_sb, wdec_sb, w_out_sb, bj_sb, bv_sb, out,
             f32, f32r, f16, Tanh, Ident):
    P = 128
    # ---- projections: enc_projT [J, T], dec_projT [J, U] ----
    enc_proj, dec_proj = [], []
    for jt in range(NJ):
        ps = psp.tile([P, FCH], f32, tag="ps")
        for dt_ in range(ND):
            nc.tensor.matmul(
                ps[:, :T],
                lhsT=wenc_sb[dt_][:, jt * P : (jt + 1) * P],
                rhs=enc_sb[dt_][:],
                start=(dt_ == 0),
                stop=(dt_ == ND - 1),
            )
        t_ = projp.tile([P, T], f32, tag=f"encproj{jt}")
        nc.scalar.copy(t_[:], ps[:, :T])
        enc_proj.append(t_)
    for jt in range(NJ):
        ps = psp.tile([P, FCH], f32, tag="ps")
        for dt_ in range(ND):
            nc.tensor.matmul(
                ps[:, :U],
                lhsT=wdec_sb[dt_][:, jt * P : (jt + 1) * P],
                rhs=dec_sb[dt_][:],
                start=(dt_ == 0),
                stop=(dt_ == ND - 1),
            )
        t_ = projp.tile([P, U], f32, tag=f"decproj{jt}")
        nc.scalar.activation(t_[:], ps[:, :U], Ident, bias=bj_sb[:, jt : jt + 1])
        dec_proj.append(t_)

    # ---- main loop over super-chunks (2 f-chunks = 1024 f each) ----
    # pre-add and tanh run double-width ([P, 1024]) to halve DVE/ACT
    # instruction counts (per-instruction SEQ/semaphore overhead is a
    # measurable per-app cost on HW); the GEMM still consumes 512-wide
    # slices (PSUM bank limit).
    for sch in range(NCHUNK // 2):
        jts = []
        for jt in range(NJ):
            pre = prep.tile([P, 2 * FCH], f32, tag="pre")
            enc_b = (
                enc_proj[jt][:, sch * 2 * TCH : (sch + 1) * 2 * TCH]
                .unsqueeze(2)
                .broadcast_to([P, 2 * TCH, U])
            )
            dec_b = dec_proj[jt][:].unsqueeze(1).broadcast_to([P, 2 * TCH, U])
            nc.vector.tensor_add(
                pre[:].rearrange("p (t u) -> p t u", t=2 * TCH), enc_b, dec_b
            )
            jtl = jointp.tile([P, 2 * FCH], f32r, tag="joint")
            nc.scalar.activation(jtl[:], pre[:], Tanh)
            jts.append(jtl)
        for half in range(2):
            ch = sch * 2 + half
            o = osbp.tile([P, NV * FCH], f16, tag="osb")
            for vt in range(NV):
                ps = psp.tile([P, FCH], f32, tag="ps")
                for jt in range(NJ):
                    nc.tensor.matmul(
                        ps[:],
                        lhsT=w_out_sb[jt][:, vt * P : (vt + 1) * P],
                        rhs=jts[jt][:, half * FCH : (half + 1) * FCH],
                        start=(jt == 0),
                        stop=(jt == NJ - 1),
                    )
                # Fused PSUM drain + b_out[v] bias (per-partition scalar),
                # alternating ACT/DVE so neither engine becomes critical.
                osl = o[:, vt * FCH : (vt + 1) * FCH]
                if vt % 2 == 0:
                    nc.scalar.activation(
                        osl, ps[:], Ident, bias=bv_sb[:, vt : vt + 1]
                    )
                else:
                    nc.vector.tensor_scalar_add(osl, ps[:], bv_sb[:, vt : vt + 1])
            # One DMA per chunk for all 8 v-tiles ([V, FCH] DRAM rectangle):
            # 8x fewer DMA instructions -> far less descriptor-generation
            # (DGE) work per application.
            nc.sync.dma_start(
                out=out[:, ch * FCH : (ch + 1) * FCH].rearrange(
                    "(v p) f -> p v f", p=P
                ),
                in_=o[:].rearrange("p (v f) -> p v f", v=NV),
            )


def _get_program():
    if "nc" not in _prog_cache:
        _prog_cache["nc"] = build_program()
    return _prog_cache["nc"]


def make_in_maps(inputs):
    enc_out = np.asarray(inputs["enc_out"], dtype=np.float32)  # (B, T, 1, D)
    dec_out = np.asarray(inputs["dec_out"], dtype=np.float32)  # (B, 1, U, D)
    W_enc = np.asarray(inputs["W_enc"], dtype=np.float32)  # (J, D)
    b_enc = np.asarray(inputs["b_enc"], dtype=np.float32)
    W_dec = np.asarray(inputs["W_dec"], dtype=np.float32)
    b_dec = np.asarray(inputs["b_dec"], dtype=np.float32)
    W_out = np.asarray(inputs["W_out"], dtype=np.float32)  # (V, J)
    b_out = np.asarray(inputs["b_out"], dtype=np.float32)

    w_enc_T = np.ascontiguousarray(W_enc.T)  # [D, J]
    w_dec_T = np.ascontiguousarray(W_dec.T)  # [D, J]
    w_out_T = np.ascontiguousarray(W_out.T)  # [J, V]
    bias_j = np.ascontiguousarray((b_enc + b_dec).reshape(J, 1))
    bias_v = np.ascontiguousarray(b_out.reshape(V, 1))

    in_maps = []
    for b in range(B):
        in_maps.append(
            {
                "enc_T": np.ascontiguousarray(enc_out[b, :, 0, :].T),  # [D, T]
                "dec_T": np.ascontiguousarray(dec_out[b, 0, :, :].T),  # [D, U]
                "w_enc_T": w_enc_T,
                "w_dec_T": w_dec_T,
                "w_out_T": w_out_T,
                "bias_j": bias_j,
                "bias_v": bias_v,
            }
        )
    return in_maps


def unpack_out(arr):
    """Device out_T [V, T*U] f16 -> full-precision (T, U, V) f32."""
    return np.ascontiguousarray(
        np.asarray(arr).astype(np.float32).reshape(V, T, U).transpose(1, 2, 0)
    )


def kernel(**inputs):
    from concourse.bass_utils import run_bass_kernel_spmd

    nc = _get_program()
    in_maps = make_in_maps(inputs)
    res = run_bass_kernel_spmd(nc, in_maps, list(range(B)))
    outs = [unpack_out(res.results[i]["out_T"]) for i in range(B)]
    return np.stack(outs, axis=0)
